# revision 48
# baseline (speedup 1.0000x reference)
"""BiDAF attention-flow layer on 8 Trainium2 NeuronCores.

Data-parallel over batch: each core processes B/8 = 8 batches.

Math (per batch b):
  s[t,j] = h[t]·w_h + u[j]·w_u + (h[t]*w_hu)·u[j] + const
  a      = softmax_j(s)            -> only needs  sj = shu + su  (row consts cancel)
  c2q    = a @ u
  bt     = softmax_t(max_j s)      -> needs  m + sh  where m = max_j(sj)
  q2c    = bt @ h
  g      = [h | c2q | h*c2q | h*q2c]

Design notes (v2, DMA-roofline oriented):
  - T=800 split into 8 chunks of 100 rows: uniform shapes, no tail cases.
  - g cols 0:200 equal h verbatim; they are assembled on the HOST during
    unsharding, so the device stores only the 600 computed columns
    (20.8MB instead of 25.9MB of HBM traffic per core).
  - h loads straight into the g working tile (cols 0:200 of each chunk),
    so no h->g copy op exists on any engine.
  - All s/c2q matmuls run in bf16 (1 cyc/row vs 4 for f32); tolerance is
    2e-2 and bf16 keeps us ~4e-3.
  - d contracted in two K-slices of 104+96 (start partitions must be
    32-aligned); the su/ones trick rides row 96 of the second slice.
  - q2c accumulated transposed ([104,2] psum, out free size 1 per mm)
    instead of as [1,200] rows (free size 200), then transposed once.
  - w_hu folded into the u-transpose drains (ACT per-partition scale);
    softmax 1/rowsum folded into the c2q psum drains and a fused
    (cps*rcp)*h scalar_tensor_tensor for the h*c2q column.
  - Loads ride the ACT HWDGE queue 2 batches ahead; stores ride SP per
    chunk-pair, so neither queue head-of-line blocks the other.
  - Engines issue strictly in order, so the whole kernel is software-
    pipelined: per slot [load(b+2) | tail(b-1) | head(b+1) | pairs(b)],
    and the pair loop runs a one-pair skew (a1/a2 vs c stages) with a
    3-deep s2 psum ring (p-transposes share the c2q psum ring).
  - GPSIMD cannot touch PSUM on real HW; every Pool op is SBUF-only.
"""
import sys

if '/opt/trn_rl_repo' not in sys.path:
    sys.path.insert(0, '/opt/trn_rl_repo')

import numpy as np

B, T, J, D = 64, 800, 50, 200
NCORES = 8
BC = B // NCORES            # batches per core
R = 100                     # rows per chunk
NCH = T // R                # 8 chunks
KA = 104                    # first K-split half of d (partition-aligned)
KB = 96                     # second half; its su/ones row sits at row 96
KB1 = KB + 1                # second half incl. the su/ones row

_cache = {}

# engine assignment knobs (sweepable): values are engine attr names
# NOTE: GPSIMD (Pool) cannot access PSUM on hardware — any knob set to
# "gpsimd" must only ever touch SBUF operands.
CFG = {
    "msh": "vector",        # msh add (reads s2 psum: DVE only)
    "ts0": "scalar",        # c2q drain chunk 0 (ACT activation w/ scale)
    "ts1": "vector",        # c2q drain chunk 1 (psum: DVE/ACT only)
    "hcq0": "gpsimd",       # h*c2q chunk 0: sbuf mul after the drain
    "hcq1": "gpsimd",       # h*c2q chunk 1: sbuf mul after the drain
    "pT2": "vector",        # pT2 psum drain (DVE/ACT only)
    "hq1": "vector",        # hq mul group 0 (reads q2cb psum: DVE)
    "hq2": "gpsimd",        # hq mul group 1 (reads q2cbs sbuf copy)
    "sumul": "gpsimd",      # su_t mul (sbuf)
    "u16": "scalar",        # u f32->bf16 copy
    "slot": "ABh",          # slot order
    "skew": 1,              # pair-loop software skew depth
}


def _split_multi_waits(nc, max_waits=1):
    """This walrus build accepts at most one sync-wait per instruction.
    For any instruction carrying more, move the extra waits onto pure-wait
    EventSemaphore carriers inserted just before it on the same engine —
    the sequencer dispatches in order, so the blocking behavior is
    identical."""
    from concourse import mybir
    import bass_rust
    n = 0
    for f in nc.m.functions:
        for blk in f.blocks:
            insts = blk.instructions
            i = 0
            while i < len(insts):
                inst = insts[i]
                si = inst.sync_info
                if si is not None and len(si.on_wait) > max_waits:
                    waits = list(si.on_wait)
                    keep = waits[-max_waits:]
                    new = []
                    for w in waits[:-max_waits]:
                        d = mybir.InstEventSemaphore(
                            name=f"{inst.name}-sw{n}", ins=[], outs=[])
                        n += 1
                        d.engine = inst.engine
                        d.sync_info = bass_rust.SyncInfo(on_wait=[w], on_update=[])
                        new.append(d)
                    inst.sync_info = bass_rust.SyncInfo(
                        on_wait=keep, on_update=list(si.on_update))
                    for j, d in enumerate(new):
                        insts.insert(i + j, d)
                    i += len(new)
                i += 1
    return n


def _build(reps=1):
    import concourse.bass as bass
    import concourse.tile as tile
    from concourse import mybir, masks
    from contextlib import ExitStack

    f32 = mybir.dt.float32
    bf16 = mybir.dt.bfloat16
    AF = mybir.ActivationFunctionType
    AX = mybir.AxisListType

    nc = bass.Bass()
    h_in = nc.declare_dram_parameter("h", [BC, T, D], f32, isOutput=False)
    u_in = nc.declare_dram_parameter("u", [BC, J, D], f32, isOutput=False)
    wh_in = nc.declare_dram_parameter("w_h", [D], f32, isOutput=False)
    wu_in = nc.declare_dram_parameter("w_u", [D], f32, isOutput=False)
    whu_in = nc.declare_dram_parameter("w_hu", [D], f32, isOutput=False)
    # device emits only the computed columns of g (c2q | h*c2q | h*q2c);
    # cols 0:200 of the final output are h verbatim and are assembled on
    # the host during unsharding — no need to round-trip them through HBM.
    g_out = nc.declare_dram_parameter("g", [BC, T, 3 * D], f32, isOutput=True)

    with tile.TileContext(nc) as tc, ExitStack() as ctx:
        singles = ctx.enter_context(tc.tile_pool(name="singles", bufs=1))
        upool = ctx.enter_context(tc.tile_pool(name="upool", bufs=3))
        repool = ctx.enter_context(tc.tile_pool(name="repool", bufs=2))
        hTpool = ctx.enter_context(tc.tile_pool(name="hTpool", bufs=CFG.get("hTbufs", 2)))
        gpool = ctx.enter_context(tc.tile_pool(name="gpool", bufs=CFG.get("gbufs", 4)))
        hqpool = ctx.enter_context(tc.tile_pool(name="hqpool", bufs=CFG.get("hqbufs", 2)))
        sm = ctx.enter_context(tc.tile_pool(name="sm", bufs=CFG.get("smbufs", 2)))
        ps_tph = ctx.enter_context(
            tc.tile_pool(name="ps_tph", bufs=CFG.get("tphbufs", 1),
                         space=bass.MemorySpace.PSUM))
        ps_sp = ctx.enter_context(
            tc.tile_pool(name="ps_sp", bufs=CFG.get("s2bufs", 3),
                         space=bass.MemorySpace.PSUM))
        ps_c2q = ctx.enter_context(
            tc.tile_pool(name="ps_c2q", bufs=2, space=bass.MemorySpace.PSUM))
        ps_small = ctx.enter_context(
            tc.tile_pool(name="ps_small", bufs=1, space=bass.MemorySpace.PSUM))

        loaded = {}

        def load_body(b):
            # loads ride the Activation HWDGE queue so stores (SP queue)
            # can never head-of-line block them
            with tc.high_priority():
                u_sb = upool.tile([J, D], f32, tag="u", name="u_sb")
                nc.scalar.dma_start(out=u_sb[:, :], in_=u_in[b, :, :])
                gt = gpool.tile([R, NCH * 3 * D], f32, tag="g", name="gt")
                nc.scalar.dma_start(
                    out=gt[:, :].rearrange("p (n x) -> p n x", x=3 * D)[:, :, 0:D],
                    in_=h_in[b, :, :].rearrange("(n p) d -> p n d", p=R))
            loaded[b] = (u_sb, gt)

        # first two batches' loads go ahead of all constant setup so the
        # big transfers start immediately
        load_body(0)
        load_body(1)

        # ---- once-per-core constants ----
        identity = singles.tile([128, 128], f32)
        masks.make_identity(nc, identity[:])
        ident16 = singles.tile([128, 128], bf16)
        masks.make_identity(nc, ident16[:])
        ones_col = singles.tile([128, 1], f32)
        nc.vector.memset(ones_col, 1.0)
        ones_row = singles.tile([1, 128], f32)
        nc.vector.memset(ones_row, 1.0)
        # row-selector matrices for the q2c broadcast: sel16[:, k*128:...]
        # has row k all-ones, the other row zero. Built with affine_select
        # (engines cannot address a lone partition 1).
        sel16 = singles.tile([2, 2 * 128], bf16, tag="sel16", name="sel16")
        nc.gpsimd.memset(sel16, 1.0)
        nc.gpsimd.affine_select(
            out=sel16[:, :], in_=sel16[:, :], compare_op=mybir.AluOpType.is_ge,
            fill=0.0, base=127, pattern=[[-1, 256]], channel_multiplier=128)
        nc.gpsimd.affine_select(
            out=sel16[:, :], in_=sel16[:, :], compare_op=mybir.AluOpType.is_ge,
            fill=0.0, base=0, pattern=[[1, 256]], channel_multiplier=-128)

        # weight halves as aligned columns (d split 104 + 96)
        whCA = singles.tile([KA, 1], f32, tag="whCA", name="whCA")
        nc.sync.dma_start(out=whCA[:, :],
                          in_=wh_in[0:KA].rearrange("(p one) -> p one", one=1))
        whCB = singles.tile([KB, 1], f32, tag="whCB", name="whCB")
        nc.sync.dma_start(out=whCB[:, :],
                          in_=wh_in[KA:D].rearrange("(p one) -> p one", one=1))
        whA16 = singles.tile([KA, 1], bf16, tag="whA16", name="whA16")
        nc.scalar.copy(out=whA16[:, :], in_=whCA[:, :])
        whB16 = singles.tile([KB1, 1], bf16, tag="whB16", name="whB16")
        nc.vector.memset(whB16, 0.0)
        nc.scalar.copy(out=whB16[0:KB, :], in_=whCB[:, :])

        whuA = singles.tile([KA, 1], f32, tag="whuA", name="whuA")
        nc.sync.dma_start(out=whuA[:, :],
                          in_=whu_in[0:KA].rearrange("(p one) -> p one", one=1))
        whuB = singles.tile([KB, 1], f32, tag="whuB", name="whuB")
        nc.sync.dma_start(out=whuB[:, :],
                          in_=whu_in[KA:D].rearrange("(p one) -> p one", one=1))
        wu_b = singles.tile([J, D], f32, tag="wu_b", name="wu_b")
        s_ap = wu_in[:]
        nc.sync.dma_start(out=wu_b[:, :], in_=bass.AP(
            tensor=s_ap.tensor, offset=s_ap.offset, ap=[[0, J], s_ap.ap[0][:]]))

        if CFG.get("warmup", 0):
            wps = ps_small.tile([128, 128], f32, tag="sm", name="warm")
            for i in range(CFG.get("warmup_n", 10)):
                nc.tensor.matmul(wps[:128, 0:128], lhsT=identity[:128, :128],
                                 rhs=identity[:128, :128], is_transpose=True,
                                 skip_group_check=True)

        # hT prologue: the su "ones" row (row 96 of the second K-half) is
        # written once per pool buffer; in-loop drains never touch rows
        # 96:104 of the B half (the memset start must be 32-aligned).
        for i in range(CFG.get("hTbufs", 2)):
            t_ = hTpool.tile([KA, 2 * T], bf16, tag="hT", name=f"hTp{i}")
            nc.gpsimd.memset(t_[KB:KA, T:2 * T], 1.0)

        views = {}

        def head_body(b):
            u_sb, gt = loaded.pop(b)
            gtv = gt[:, :].rearrange("p (n x) -> p n x", x=3 * D)

            # ---- u-side prep ----
            u16 = upool.tile([J, D], bf16, tag="u16", name="u16")
            if CFG["u16"] == "scalar":
                nc.scalar.copy(out=u16[:, :], in_=u_sb[:, :])
            else:
                getattr(nc, CFG["u16"]).tensor_copy(out=u16[:, :], in_=u_sb[:, :])
            su_t = upool.tile([J, D], f32, tag="su_t", name="su_t")
            getattr(nc, CFG["sumul"]).tensor_mul(out=su_t[:, :], in0=u_sb[:, :], in1=wu_b[:, :])
            su_col = sm.tile([J, 1], f32, tag="su")
            nc.vector.reduce_sum(out=su_col[:, :], in_=su_t[:, :], axis=AX.X)

            tpu = ps_small.tile([KA, 2 * J], bf16, tag="sm", name="tpu")
            nc.tensor.transpose(tpu[:KA, 0:J], u16[:J, 0:KA], ident16[:J, :J])
            nc.tensor.transpose(tpu[:KB, J:2 * J], u16[:J, KA:D],
                                ident16[:J, :J])
            # drains fold in w_hu (per-partition scale): re = (u^T) * w_hu
            reA = repool.tile([KA, J + 1], bf16, tag="reA", name="reA")
            reB = repool.tile([KB1, J + 1], bf16, tag="reB", name="reB")
            nc.scalar.activation(out=reA[:KA, 0:J], in_=tpu[:KA, 0:J],
                                 func=AF.Copy, scale=whuA[:, 0:1])
            nc.scalar.activation(out=reB[0:KB, 0:J], in_=tpu[:KB, J:2 * J],
                                 func=AF.Copy, scale=whuB[:, 0:1])
            nc.gpsimd.tensor_copy(out=reA[:KA, J:J + 1], in_=whA16[:, :])
            nc.gpsimd.tensor_copy(out=reB[0:KB1, J:J + 1], in_=whB16[:, :])
            sutp = ps_small.tile([1, J], f32, tag="sm", name="sutp")
            nc.tensor.transpose(sutp[:1, :J], su_col[:J, :1], identity[:J, :J])
            nc.scalar.copy(out=reB[KB:KB1, 0:J], in_=sutp[:1, :J])

            # ---- h transpose: hT [104, 2*800] bf16, B-half row 96 = ones ----
            hT = hTpool.tile([KA, 2 * T], bf16, tag="hT", name="hT")
            for kp in range(4):
                tph = ps_tph.tile([KA, 4 * R], f32, tag="tph", name="tph")
                t0 = 2 * kp * R
                for ci in range(2):
                    c = 2 * kp + ci
                    nc.tensor.matmul(
                        tph[:KA, ci * R:(ci + 1) * R],
                        lhsT=gtv[:, c, 0:KA],
                        rhs=identity[:R, :R], is_transpose=True,
                        skip_group_check=True)
                    nc.tensor.matmul(
                        tph[:KB, (2 + ci) * R:(3 + ci) * R],
                        lhsT=gtv[:, c, KA:D],
                        rhs=identity[:R, :R], is_transpose=True,
                        skip_group_check=True)
                nc.scalar.copy(out=hT[:KA, t0:t0 + 2 * R],
                               in_=tph[:KA, 0:2 * R])
                nc.scalar.copy(out=hT[:KB, T + t0:T + t0 + 2 * R],
                               in_=tph[:KB, 2 * R:4 * R])

            msh_all = sm.tile([R, NCH], f32, tag="msh", name="msh_all")
            views[b] = (u_sb, u16, gt, gtv, hT, reA, reB, msh_all)

        def pairloop_body(b):
            u_sb, u16, gt, gtv, hT, reA, reB, msh_all = views[b]
            st = {}

            def stage_a1(kp):
                # s matmuls + row max + msh + exp
                s2 = ps_sp.tile([R, 2 * (J + 1)], f32, tag="s2", name="s2")
                for ci in range(2):
                    c = 2 * kp + ci
                    t0 = c * R
                    so = ci * (J + 1)
                    nc.tensor.matmul(s2[:R, so:so + J + 1],
                                     lhsT=hT[0:KA, t0:t0 + R],
                                     rhs=reA[0:KA, :], start=True, stop=False)
                    nc.tensor.matmul(s2[:R, so:so + J + 1],
                                     lhsT=hT[0:KB1, T + t0:T + t0 + R],
                                     rhs=reB[0:KB1, :], start=False, stop=True)
                s2v = s2[:R, :].rearrange("p (k j) -> p k j", j=J + 1)
                m2 = sm.tile([R, 2], f32, tag="m")
                nc.vector.reduce_max(out=m2[:R, :], in_=s2v[:, :, 0:J], axis=AX.X)
                getattr(nc, CFG["msh"]).tensor_add(
                    out=msh_all[:R, 2 * kp:2 * kp + 2].rearrange(
                        "p (k one) -> p k one", one=1),
                    in0=m2[:R, :].rearrange("p (k one) -> p k one", one=1),
                    in1=s2v[:, :, J:J + 1])
                p2 = sm.tile([R, 2 * J], bf16, tag="p2", name="p2")
                nc.scalar.activation(
                    out=p2[:R, :].rearrange("p (k j) -> p k j", j=J),
                    in_=s2v[:, :, 0:J], func=AF.Exp)
                st[kp] = [p2]

            def stage_a2(kp):
                # row sums + reciprocal (split off so stage_c of the
                # previous pair can issue on DVE before rs2 blocks it)
                (p2,) = st[kp]
                rs2 = sm.tile([R, 2], f32, tag="rs")
                nc.vector.reduce_sum(
                    out=rs2[:R, :],
                    in_=p2[:R, :].rearrange("p (k j) -> p k j", j=J), axis=AX.X)
                rcp2 = sm.tile([R, 2], f32, tag="rcp")
                nc.vector.reciprocal(out=rcp2[:R, :], in_=rs2[:R, :])
                st[kp] = [p2, rcp2]

            def stage_c(kp):
                # transpose p, c2q matmul, drains, h*c2q, store
                p2, rcp2 = st.pop(kp)
                tpp = ps_c2q.tile([J, 2 * R], bf16, tag="cps", name="tpp")
                for ci in range(2):
                    nc.tensor.transpose(tpp[:J, ci * R:(ci + 1) * R],
                                        p2[:R, ci * J:(ci + 1) * J],
                                        ident16[:R, :R])
                pT2 = sm.tile([J, 2 * R], bf16, tag="pT", name="pT2")
                if CFG["pT2"] == "scalar":
                    nc.scalar.copy(out=pT2[:J, :], in_=tpp[:J, :])
                else:
                    getattr(nc, CFG["pT2"]).tensor_copy(out=pT2[:J, :], in_=tpp[:J, :])
                cps = ps_c2q.tile([R, 2 * D], f32, tag="cps", name="cps")
                for ci in range(2):
                    nc.tensor.matmul(cps[:R, ci * D:(ci + 1) * D],
                                     lhsT=pT2[:J, ci * R:(ci + 1) * R],
                                     rhs=u16[:J, :], start=True, stop=True)
                for ci in range(2):
                    c = 2 * kp + ci
                    # c2q drain with the softmax 1/rowsum folded in
                    tseng = CFG["ts0"] if ci == 0 else CFG["ts1"]
                    if tseng == "scalar":
                        nc.scalar.activation(
                            out=gtv[:, c, D:2 * D],
                            in_=cps[:R, ci * D:(ci + 1) * D],
                            func=AF.Copy, scale=rcp2[:R, ci:ci + 1])
                    else:
                        getattr(nc, tseng).tensor_scalar_mul(
                            out=gtv[:, c, D:2 * D],
                            in0=cps[:R, ci * D:(ci + 1) * D],
                            scalar1=rcp2[:R, ci:ci + 1])
                    hcq = CFG["hcq0"] if ci == 0 else CFG["hcq1"]
                    if hcq == "stt":
                        # h*c2q fused on DVE: (cps*rcp)*h, psum-legal,
                        # independent of the drain
                        nc.vector.scalar_tensor_tensor(
                            out=gtv[:, c, 2 * D:3 * D],
                            in0=cps[:R, ci * D:(ci + 1) * D],
                            scalar=rcp2[:R, ci:ci + 1],
                            in1=gtv[:, c, 0:D],
                            op0=mybir.AluOpType.mult, op1=mybir.AluOpType.mult)
                    else:
                        # h*c2q from the drained sbuf copy (Pool-legal)
                        getattr(nc, hcq).tensor_mul(
                            out=gtv[:, c, 2 * D:3 * D],
                            in0=gtv[:, c, 0:D], in1=gtv[:, c, D:2 * D])
                nc.sync.dma_start(
                    out=g_out[b, 2 * kp * R:(2 * kp + 2) * R, 0:2 * D].rearrange(
                        "(c p) x -> p c x", p=R),
                    in_=gtv[:, 2 * kp:2 * kp + 2, D:3 * D])

            if CFG.get("skew", 2) == 2:
                # two-pair software skew
                stage_a1(0)
                stage_a2(0)
                stage_a1(1)
                stage_a2(1)
                stage_c(0)
                stage_a1(2)
                stage_a2(2)
                stage_c(1)
                stage_a1(3)
                stage_a2(3)
                stage_c(2)
                stage_c(3)
            else:
                # one-pair software skew
                stage_a1(0)
                stage_a2(0)
                stage_a1(1)
                stage_c(0)
                stage_a2(1)
                stage_a1(2)
                stage_c(1)
                stage_a2(2)
                stage_a1(3)
                stage_c(2)
                stage_a2(3)
                stage_c(3)

        def tail_bodyA(b):
            # e = exp(m+sh), transposed y accumulation, and the sum chain
            u_sb, u16, gt, gtv, hT, reA, reB, msh_all = views[b]
            e_all = sm.tile([R, NCH], f32, tag="e", name="e_all")
            nc.scalar.activation(out=e_all[:R, :], in_=msh_all[:R, :], func=AF.Exp)
            yT = ps_small.tile([KA, 2], f32, tag="sm", name="yT")
            for k in range(2):
                kn = KA if k == 0 else KB
                for c in range(NCH):
                    nc.tensor.matmul(yT[:kn, k:k + 1],
                                     lhsT=gtv[:, c, k * KA:k * KA + kn],
                                     rhs=e_all[:R, c:c + 1],
                                     start=(c == 0), stop=(c == NCH - 1))
            yT_sb = sm.tile([KA, 2], f32, tag="yT_sb")
            nc.vector.tensor_copy(out=yT_sb[:KA, :], in_=yT[:KA, :])
            ecol = sm.tile([R, 1], f32, tag="ecol")
            nc.vector.reduce_sum(out=ecol[:R, :], in_=e_all[:R, :], axis=AX.X)
            views[b] = (u_sb, u16, gt, gtv, hT, reA, reB, msh_all, yT_sb, ecol)

        def tail_bodyB(b):
            # q2c normalize/broadcast, h*q2c, hq store
            u_sb, u16, gt, gtv, hT, reA, reB, msh_all, yT_sb, ecol = views.pop(b)
            Stot = ps_small.tile([1, 1], f32, tag="sm", name="Stot")
            nc.tensor.matmul(Stot[:1, :], lhsT=ecol[:R, :], rhs=ones_col[:R, :],
                             start=True, stop=True)
            Sinv = sm.tile([1, 1], f32, tag="Sinv")
            nc.vector.reciprocal(out=Sinv[:1, :], in_=Stot[:1, :])
            Sb2 = ps_small.tile([2, 1], f32, tag="sm", name="Sb2")
            nc.tensor.matmul(Sb2[:2, :], lhsT=ones_row[:1, 0:2], rhs=Sinv[:1, :],
                             start=True, stop=True)
            Sb2s = sm.tile([2, 1], f32, tag="Sb2s")
            nc.scalar.copy(out=Sb2s[:2, :], in_=Sb2[:2, :])
            tpy = ps_small.tile([2, KA], f32, tag="sm", name="tpy")
            nc.tensor.transpose(tpy[:2, :KA], yT_sb[:KA, 0:2], identity[:KA, :KA])
            q2c2 = sm.tile([2, KA], bf16, tag="q2c2")
            nc.vector.tensor_scalar_mul(out=q2c2[:2, :], in0=tpy[:2, :KA],
                                        scalar1=Sb2s[:2, :])
            q2cb = ps_small.tile([R, D], f32, tag="sm", name="q2cb")
            for k in range(2):
                kn = KA if k == 0 else KB
                nc.tensor.matmul(q2cb[:R, k * KA:k * KA + kn],
                                 lhsT=sel16[:2, k * 128:k * 128 + R],
                                 rhs=q2c2[:2, 0:kn],
                                 start=True, stop=True)
            q2cbs = sm.tile([R, D], f32, tag="q2cbs")
            nc.scalar.copy(out=q2cbs[:R, :], in_=q2cb[:R, :])
            hq = hqpool.tile([R, NCH * D], f32, tag="hq", name="hq")
            hqv = hq[:, :].rearrange("p (n d) -> p n d", d=D)
            for g4 in range(2):
                eng_name = CFG["hq1"] if g4 == 0 else CFG["hq2"]
                src_ap = q2cb[:R, :] if eng_name == "vector" else q2cbs[:R, :]
                b4 = bass.AP(tensor=src_ap.tensor, offset=src_ap.offset,
                             ap=[src_ap.ap[0][:], [0, 4], src_ap.ap[1][:]])
                getattr(nc, eng_name).tensor_mul(
                    out=hqv[:, 4 * g4:4 * g4 + 4, :],
                    in0=gtv[:, 4 * g4:4 * g4 + 4, 0:D], in1=b4)
            nc.sync.dma_start(
                out=g_out[b, :, 2 * D:3 * D].rearrange("(n p) d -> p n d", p=R),
                in_=hqv[:, :, :])

        def run_all():
            # software pipeline, one slot per batch:
            #   load(b+2) | tailA(b-1) | head(b+1) | tailB(b-1) | pairloop(b)
            # tailA's PE accumulation runs right after e; head(b+1)'s
            # transposes fill PE while tailB's cross-engine chain resolves.
            order = CFG.get("slot", "AhB")
            head_body(0)
            for b in range(BC):
                if b + 2 < BC:
                    load_body(b + 2)
                if order == "AhB":
                    if b > 0:
                        tail_bodyA(b - 1)
                    if b + 1 < BC:
                        head_body(b + 1)
                    if b > 0:
                        tail_bodyB(b - 1)
                elif order == "hAB":
                    if b + 1 < BC:
                        head_body(b + 1)
                    if b > 0:
                        tail_bodyA(b - 1)
                        tail_bodyB(b - 1)
                elif order == "ABh":
                    if b > 0:
                        tail_bodyA(b - 1)
                        tail_bodyB(b - 1)
                    if b + 1 < BC:
                        head_body(b + 1)
                pairloop_body(b)
            tail_bodyA(BC - 1)
            tail_bodyB(BC - 1)

        if reps == 1:
            run_all()
        else:
            with tc.For_i(0, reps, 1):
                run_all()

    return nc


def kernel(h, u, w_h, b_h, w_u, b_u, w_hu, b_hu):
    from concourse.bass_utils import run_bass_kernel_spmd

    if "nc" not in _cache:
        nc = _build()
        _split_multi_waits(nc)
        _cache["nc"] = nc
    nc = _cache["nc"]

    h = np.ascontiguousarray(h, dtype=np.float32)
    u = np.ascontiguousarray(u, dtype=np.float32)
    w_h = np.ascontiguousarray(w_h, dtype=np.float32)
    w_u = np.ascontiguousarray(w_u, dtype=np.float32)
    w_hu = np.ascontiguousarray(w_hu, dtype=np.float32)

    core_ids = list(range(NCORES))
    in_maps = []
    for i in core_ids:
        in_maps.append({
            "h": h[i * BC:(i + 1) * BC],
            "u": u[i * BC:(i + 1) * BC],
            "w_h": w_h,
            "w_u": w_u,
            "w_hu": w_hu,
        })
    res = run_bass_kernel_spmd(nc, in_maps, core_ids)
    _cache["last_results"] = res
    out = np.empty((B, T, 4 * D), dtype=np.float32)
    out[:, :, 0:D] = h  # identity pass-through column of g
    out[:, :, D:4 * D] = np.concatenate(
        [res.results[i]["g"] for i in core_ids], axis=0)
    return out
